# revision 14
# baseline (speedup 1.0000x reference)
"""Trainium2 Bass kernel for nn_Encoder (dense MLP 6->8->4->2->1 + softplus).

v3: pure data parallel over 8 NeuronCores. Block-diagonal full-array
matmuls (16 rows per PE column), with:
  - L1 bias folded into the matmul via a ones partition (K=97), so the z1
    evacuation is a single-op relu;
  - a post-pass that deletes redundant LDWEIGHTS (walrus's own ldw-opt
    crashes in this container), cutting 11 -> 6 weight loads/supertile;
  - final bias b9 + softplus on the host in float64 (device ships raw z4
    as bf16), removing all ACT table functions;
  - PSUM->SBUF evacuations balanced across DVE and ACT.
"""

import os
import sys

sys.path.insert(0, "/opt/trn_rl_repo")

import numpy as np

import concourse.bass as bass
import concourse.mybir as mybir
import concourse.tile as tile
from concourse.bass_utils import run_bass_kernel_spmd

# ---------------------------------------------------------------- geometry
N_CORES = 8
N_ROWS = 4194304
ROWS_PER_CORE = N_ROWS // N_CORES          # 524288
G = 16                                      # rows per PE column
ST_COLS = 2048                              # columns per supertile
ROWS_ST = ST_COLS * G                       # 32768 rows per supertile
N_ST = ROWS_PER_CORE // ROWS_ST             # 16 supertiles per core
FD = 512
BF16 = mybir.dt.bfloat16
F32 = mybir.dt.float32

_MAX_WAITS = int(os.environ.get("KMAXW", "1"))


def _split_multi_waits(nc, max_waits=_MAX_WAITS):
    ctr = 0
    for f in nc.m.functions:
        for bb in f.blocks:
            out = []
            for inst in bb.instructions:
                mw = 1 if ("Dma" in inst.opcode or "DMA" in inst.opcode
                           or "Trigger" in inst.opcode) else max_waits
                si = getattr(inst, "sync_info", None)
                if si is not None and si.on_wait and len(si.on_wait) > mw:
                    waits = list(si.on_wait)
                    split = len(waits) - mw
                    for i in range(0, split, max_waits):
                        nop = mybir.InstNoOp(
                            name=f"waitsplit-{ctr}", ins=[], outs=[]
                        )
                        ctr += 1
                        nop.engine = inst.engine
                        nop.sync_info = mybir.SyncInfo(
                            on_wait=waits[i : min(i + max_waits, split)],
                            on_update=[],
                        )
                        out.append(nop)
                    inst.sync_info = mybir.SyncInfo(
                        on_wait=waits[split:], on_update=list(si.on_update)
                    )
                out.append(inst)
            bb.instructions[:] = out


def _ldw_key(inst):
    """Identity of a stationary load: weights AP + array placement."""
    return (
        repr(inst.ins),
        repr(getattr(inst, "tile_position", None)),
        repr(getattr(inst, "tile_size", None)),
        repr(getattr(inst, "perf_mode", None)),
    )


def _dedupe_ldweights(nc):
    """Drop Ldweights that reload the stationary already resident at the
    same array position; their sync info migrates to the next instruction
    so ordering/semaphore semantics are preserved."""
    removed = 0
    for f in nc.m.functions:
        for bb in f.blocks:
            out = []
            last = None
            pend_w, pend_u = [], []
            for inst in bb.instructions:
                if inst.opcode == "Ldweights":
                    k = _ldw_key(inst)
                    if k == last:
                        si = getattr(inst, "sync_info", None)
                        if si is not None:
                            pend_w.extend(si.on_wait or [])
                            pend_u.extend(si.on_update or [])
                        removed += 1
                        continue
                    last = k
                elif inst.opcode == "Matmult":
                    pass  # matmuls don't disturb the stationary
                if pend_w or pend_u:
                    si = getattr(inst, "sync_info", None)
                    ow = list(si.on_wait) if si is not None else []
                    ou = list(si.on_update) if si is not None else []
                    inst.sync_info = mybir.SyncInfo(
                        on_wait=ow + pend_w, on_update=ou + pend_u
                    )
                    pend_w, pend_u = [], []
                out.append(inst)
            assert not pend_w and not pend_u
            bb.instructions[:] = out
    return removed


TRACE = os.environ.get("KERNEL_TRACE", "0") == "1"
LAST_RESULTS = None


def _register_ntff_hook():
    """The image's antenv lacks axon_hooks; inject it and register the ctypes
    NTFF profile hook so run_bass_kernel_spmd(trace=True) works under axon."""
    import types

    if "antenv.axon_hooks" not in sys.modules:
        mod = types.ModuleType("antenv.axon_hooks")
        mod._hook = None

        def set_axon_ntff_profile_hook(h, _mod=mod):
            _mod._hook = h

        def get_axon_ntff_profile_hook(_mod=mod):
            return _mod._hook

        mod.set_axon_ntff_profile_hook = set_axon_ntff_profile_hook
        mod.get_axon_ntff_profile_hook = get_axon_ntff_profile_hook
        sys.modules["antenv.axon_hooks"] = mod
        import antenv

        antenv.axon_hooks = mod
    mod = sys.modules["antenv.axon_hooks"]
    if mod.get_axon_ntff_profile_hook() is None:
        try:
            from trn_agent_boot.trn_boot import _ntff_profile_via_ctypes

            mod.set_axon_ntff_profile_hook(
                _ntff_profile_via_ctypes("/opt/axon/libaxon_pjrt.so")
            )
        except Exception:
            pass


# ---------------------------------------------------------------- program
def build_program(n_st=N_ST, split_waits=True, warmup=28):
    """One SPMD NeuronCore program; all 8 cores run it on their own shard."""
    nc = bass.Bass("TRN2", target_bir_lowering=False, debug=False,
                   num_devices=N_CORES)

    xb = nc.dram_tensor("xb", [n_st, 96, ST_COLS], BF16,
                        kind="ExternalInput").ap()
    w1 = nc.dram_tensor("w1blk", [97, 128], BF16, kind="ExternalInput").ap()
    w2 = nc.dram_tensor("w2blk", [128, 64], BF16, kind="ExternalInput").ap()
    w3 = nc.dram_tensor("w3blk", [128, 64], BF16, kind="ExternalInput").ap()
    w4 = nc.dram_tensor("w4blk", [128, 64], BF16, kind="ExternalInput").ap()
    bv = nc.dram_tensor("bvecs", [128, 2], F32, kind="ExternalInput").ap()
    out = nc.dram_tensor("out", [n_st // 2, 128, FD], BF16,
                         kind="ExternalOutput").ap()

    Relu = mybir.ActivationFunctionType.Relu
    ADD = mybir.AluOpType.add
    MAX = mybir.AluOpType.max

    with tile.TileContext(nc) as tc:
        with (
            tc.tile_pool(name="consts", bufs=1) as cpool,
            tc.tile_pool(name="xin", bufs=1) as xpool,
            tc.tile_pool(name="zr", bufs=1) as zrpool,
            tc.tile_pool(name="ps", bufs=1, space="PSUM") as pspool,
        ):
            # fixed SBUF buffers
            xts = [xpool.tile([128, ST_COLS], BF16, tag=f"x{k}", name=f"x{k}")
                   for k in range(4)]
            z1rA = [zrpool.tile([128, 1024], BF16, tag=f"z1rA{k}",
                                name=f"z1rA{k}") for k in range(2)]
            z1rB = [zrpool.tile([128, 1024], BF16, tag=f"z1rB{k}",
                                name=f"z1rB{k}") for k in range(2)]
            z2r = [zrpool.tile([128, 1024], BF16, tag=f"z2r{k}",
                               name=f"z2r{k}") for k in range(2)]
            z3r = [zrpool.tile([128, 512], BF16, tag=f"z3r{k}",
                               name=f"z3r{k}") for k in range(2)]
            z4r = [zrpool.tile([128, 512], BF16, tag=f"z4r{k}",
                               name=f"z4r{k}") for k in range(2)]

            # fixed PSUM tiles (8 banks exactly)
            z1a = pspool.tile([128, 1024], F32, tag="z1a", name="z1a")
            z1b = pspool.tile([128, 1024], F32, tag="z1b", name="z1b")
            z2p = pspool.tile([128, 1024], F32, tag="z2p", name="z2p")
            z3p = pspool.tile([128, 512], F32, tag="z3p", name="z3p")
            z4p = pspool.tile([128, 512], F32, tag="z4p", name="z4p")

            # ones partitions (L1 bias row): just partition 96 of each
            # x buffer (16-aligned, so gpsimd can address it)
            for xt in xts:
                nc.gpsimd.memset(xt[96:97, :], 1.0)

            def load_x(s):
                xt = xts[s % 4]
                nc.sync.dma_start(xt[0:96, 0:1024], xb[s, :, 0:1024])
                nc.sync.dma_start(xt[0:96, 1024:2048], xb[s, :, 1024:2048])
                return xt

            for s in range(min(3, n_st)):
                load_x(s)
            w1t = cpool.tile([97, 128], BF16, tag="w1")
            nc.sync.dma_start(w1t[:], w1[:])
            w2t = cpool.tile([128, 64], BF16, tag="w2")
            nc.sync.dma_start(w2t[:], w2[:])
            w3t = cpool.tile([128, 64], BF16, tag="w3")
            nc.sync.dma_start(w3t[:], w3[:])
            w4t = cpool.tile([128, 64], BF16, tag="w4")
            nc.sync.dma_start(w4t[:], w4[:])
            bvt = cpool.tile([128, 2], F32, tag="bv")
            nc.sync.dma_start(bvt[:], bv[:])
            b7v, b8v = bvt[:, 0:1], bvt[:, 1:2]

            # PE warmup against the HAM clock gate, into z1a's first bank
            if warmup:
                wscr = cpool.tile([32, 640], BF16, tag="wscr")
                nc.gpsimd.memset(wscr[:], 0.0)
                for _ in range(warmup):
                    nc.tensor.matmul(z1a[0:128, 0:512], wscr[0:32, 0:128],
                                     wscr[0:32, 128:640], start=True,
                                     stop=True)

            for it in range(n_st + 3):
                # L1 for supertile s1: 4 matmuls, one shared stationary
                if it < n_st:
                    s1 = it
                    if s1 + 3 < n_st:
                        load_x(s1 + 3)
                    xt = xts[s1 % 4]
                    for c in range(4):
                        zt, c0 = (z1a, c * 512) if c < 2 else (z1b,
                                                               (c - 2) * 512)
                        nc.tensor.matmul(
                            zt[:, c0 : c0 + 512],
                            w1t[:],
                            xt[0:97, 512 * c : 512 * c + 512],
                            start=True, stop=True,
                        )
                    if s1 == n_st - 1:
                        # fine-grained for the drain tail: L2 chunks start
                        # as soon as their 512-col slice is evacuated
                        for h in range(2):
                            nc.vector.tensor_scalar_max(
                                z1rA[s1 % 2][:, h * 512 : h * 512 + 512],
                                z1a[:, h * 512 : h * 512 + 512], 0.0)
                            nc.scalar.activation(
                                z1rB[s1 % 2][:, h * 512 : h * 512 + 512],
                                z1b[:, h * 512 : h * 512 + 512], Relu)
                    else:
                        nc.vector.tensor_scalar_max(z1rA[s1 % 2][:], z1a[:],
                                                    0.0)
                        nc.scalar.activation(z1rB[s1 % 2][:], z1b[:], Relu)

                # L2 for supertile s2 (j outer so identical LDWs are
                # consecutive and dedupe)
                if 0 <= it - 1 < n_st:
                    s2 = it - 1
                    vA, vB = z1rA[s2 % 2], z1rB[s2 % 2]
                    for j in range(2):
                        for half in range(2):
                            src = vA if half == 0 else vB
                            nc.tensor.matmul(
                                z2p[j * 64 : j * 64 + 64,
                                    half * 512 : half * 512 + 512],
                                w2t[:],
                                src[:, j * 512 : j * 512 + 512],
                                start=True, stop=True,
                            )
                    if s2 == n_st - 1:
                        for h in range(2):
                            nc.scalar.activation(
                                z2r[s2 % 2][:, h * 512 : h * 512 + 512],
                                z2p[:, h * 512 : h * 512 + 512], Relu,
                                bias=b7v, scale=1.0)
                    else:
                        nc.scalar.activation(z2r[s2 % 2][:], z2p[:], Relu,
                                             bias=b7v, scale=1.0)

                # L3 for supertile s3
                if 0 <= it - 2 < n_st:
                    s3 = it - 2
                    v2 = z2r[s3 % 2]
                    for u in range(2):
                        nc.tensor.matmul(
                            z3p[u * 64 : u * 64 + 64, :],
                            w3t[:],
                            v2[:, u * 512 : u * 512 + 512],
                            start=True, stop=True,
                        )
                    nc.vector.tensor_scalar(z3r[s3 % 2][:], z3p[:], b8v, 0.0,
                                            ADD, MAX)

                # L4 for supertile s4; supertile pairs share one PSUM bank
                if 0 <= it - 3 < n_st:
                    s4 = it - 3
                    half4 = (s4 % 2) * 64
                    nc.tensor.matmul(z4p[half4 : half4 + 64, :], w4t[:],
                                     z3r[s4 % 2][:], start=True, stop=True)
                    if s4 % 2 == 1:
                        zo = z4r[(s4 // 2) % 2]
                        nc.vector.tensor_copy(zo[:], z4p[:])
                        nc.sync.dma_start(out[s4 // 2], zo[:])

    _dedupe_ldweights(nc)
    if split_waits:
        _split_multi_waits(nc)
    return nc


# ---------------------------------------------------------------- host side
def _block_weights(W1, b1, W7, W8, W9):
    w1blk = np.zeros((97, 128), np.float32)
    for r in range(16):
        w1blk[r * 6 : r * 6 + 6, r * 8 : r * 8 + 8] = W1.T
    w1blk[96, :] = np.tile(b1, 16)
    w2blk = np.zeros((128, 64), np.float32)
    for r in range(16):
        w2blk[r * 8 : r * 8 + 8, r * 4 : r * 4 + 4] = W7.T
    w3blk = np.zeros((128, 64), np.float32)
    for h in range(2):
        for r in range(16):
            w3blk[h * 64 + r * 4 : h * 64 + r * 4 + 4,
                  h * 32 + r * 2 : h * 32 + r * 2 + 2] = W8.T
    w4blk = np.zeros((128, 64), np.float32)
    for q in range(2):
        for h in range(2):
            for r in range(16):
                w4blk[q * 64 + h * 32 + r * 2 : q * 64 + h * 32 + r * 2 + 2,
                      q * 32 + h * 16 + r] = W9.T[:, 0]
    return w1blk, w2blk, w3blk, w4blk


def kernel(x, W1, b1, W7, b7, W8, b8, W9, b9):
    import ml_dtypes

    x = np.ascontiguousarray(np.asarray(x, dtype=np.float32))
    W1, b1 = np.asarray(W1, np.float32), np.asarray(b1, np.float32)
    W7, b7 = np.asarray(W7, np.float32), np.asarray(b7, np.float32)
    W8, b8 = np.asarray(W8, np.float32), np.asarray(b8, np.float32)
    W9, b9 = np.asarray(W9, np.float32), np.asarray(b9, np.float32)

    bf = ml_dtypes.bfloat16
    w1blk, w2blk, w3blk, w4blk = _block_weights(W1, b1, W7, W8, W9)
    p = np.arange(128)
    bvecs = np.stack([b7[p % 4], b8[p % 2]], axis=1).astype(np.float32)

    # [N,6] -> per core [n_st, 96, st_cols]: block layout, partition = r*6+k
    xbh = (
        x.reshape(N_CORES, N_ST, ST_COLS, G, 6)
        .transpose(0, 1, 3, 4, 2)
        .reshape(N_CORES, N_ST, 96, ST_COLS)
        .astype(bf)
    )
    xbh = np.ascontiguousarray(xbh)

    nc = build_program()
    in_maps = [
        {
            "xb": xbh[c],
            "w1blk": w1blk.astype(bf),
            "w2blk": w2blk.astype(bf),
            "w3blk": w3blk.astype(bf),
            "w4blk": w4blk.astype(bf),
            "bvecs": bvecs,
        }
        for c in range(N_CORES)
    ]
    kwargs = {}
    if TRACE:
        _register_ntff_hook()
        kwargs["trace"] = True
    res = run_bass_kernel_spmd(nc, in_maps, list(range(N_CORES)), **kwargs)
    global LAST_RESULTS
    LAST_RESULTS = res

    # out[c] is [8, 128, 512] raw z4 (bf16);
    # row = ((((P*2+so)*2+q)*2+h)*512+c)*16+r. Host: +b9, softplus, fp32.
    outs = []
    for c in range(N_CORES):
        arr = np.asarray(res.results[c]["out"]).astype(np.float32)
        arr = (
            arr.reshape(N_ST // 2, 2, 2, 2, G, FD)
            .transpose(0, 1, 2, 3, 5, 4)
            .reshape(ROWS_PER_CORE)
        )
        outs.append(arr)
    z4 = np.concatenate(outs).astype(np.float64) + float(b9[0])
    y = np.logaddexp(0.0, z4).astype(np.float32)
    return np.ascontiguousarray(y.reshape(N_ROWS, 1))


# revision 15
# speedup vs baseline: 1.2316x; 1.2316x over previous
"""Trainium2 Bass kernel for nn_Encoder (dense MLP 6->8->4->2->1 + softplus).

v3: pure data parallel over 8 NeuronCores. Block-diagonal full-array
matmuls (16 rows per PE column), with:
  - L1 bias folded into the matmul via a ones partition (K=97), so the z1
    evacuation is a single-op relu;
  - a post-pass that deletes redundant LDWEIGHTS (walrus's own ldw-opt
    crashes in this container), cutting 11 -> 6 weight loads/supertile;
  - final bias b9 + softplus on the host in float64 (device ships raw z4
    as bf16), removing all ACT table functions;
  - PSUM->SBUF evacuations balanced across DVE and ACT.
"""

import os
import sys

sys.path.insert(0, "/opt/trn_rl_repo")

import numpy as np

import concourse.bass as bass
import concourse.mybir as mybir
import concourse.tile as tile
from concourse.bass_utils import run_bass_kernel_spmd

# ---------------------------------------------------------------- geometry
N_CORES = 8
N_ROWS = 4194304
ROWS_PER_CORE = N_ROWS // N_CORES          # 524288
G = 16                                      # rows per PE column
ST_COLS = 2048                              # columns per supertile
ROWS_ST = ST_COLS * G                       # 32768 rows per supertile
N_ST = ROWS_PER_CORE // ROWS_ST             # 16 supertiles per core
FD = 512
BF16 = mybir.dt.bfloat16
F32 = mybir.dt.float32

_MAX_WAITS = int(os.environ.get("KMAXW", "1"))


def _split_multi_waits(nc, max_waits=_MAX_WAITS):
    ctr = 0
    for f in nc.m.functions:
        for bb in f.blocks:
            out = []
            for inst in bb.instructions:
                mw = 1 if ("Dma" in inst.opcode or "DMA" in inst.opcode
                           or "Trigger" in inst.opcode) else max_waits
                si = getattr(inst, "sync_info", None)
                if si is not None and si.on_wait and len(si.on_wait) > mw:
                    waits = list(si.on_wait)
                    split = len(waits) - mw
                    for i in range(0, split, max_waits):
                        nop = mybir.InstNoOp(
                            name=f"waitsplit-{ctr}", ins=[], outs=[]
                        )
                        ctr += 1
                        nop.engine = inst.engine
                        nop.sync_info = mybir.SyncInfo(
                            on_wait=waits[i : min(i + max_waits, split)],
                            on_update=[],
                        )
                        out.append(nop)
                    inst.sync_info = mybir.SyncInfo(
                        on_wait=waits[split:], on_update=list(si.on_update)
                    )
                out.append(inst)
            bb.instructions[:] = out


def _ldw_key(inst):
    """Identity of a stationary load: weights AP + array placement."""
    return (
        repr(inst.ins),
        repr(getattr(inst, "tile_position", None)),
        repr(getattr(inst, "tile_size", None)),
        repr(getattr(inst, "perf_mode", None)),
    )


def _dedupe_ldweights(nc):
    """Drop Ldweights that reload the stationary already resident at the
    same array position; their sync info migrates to the next instruction
    so ordering/semaphore semantics are preserved."""
    removed = 0
    for f in nc.m.functions:
        for bb in f.blocks:
            out = []
            last = None
            pend_w, pend_u = [], []
            for inst in bb.instructions:
                if inst.opcode == "Ldweights":
                    k = _ldw_key(inst)
                    if k == last:
                        si = getattr(inst, "sync_info", None)
                        if si is not None:
                            pend_w.extend(si.on_wait or [])
                            pend_u.extend(si.on_update or [])
                        removed += 1
                        continue
                    last = k
                elif inst.opcode == "Matmult":
                    pass  # matmuls don't disturb the stationary
                if pend_w or pend_u:
                    si = getattr(inst, "sync_info", None)
                    ow = list(si.on_wait) if si is not None else []
                    ou = list(si.on_update) if si is not None else []
                    inst.sync_info = mybir.SyncInfo(
                        on_wait=ow + pend_w, on_update=ou + pend_u
                    )
                    pend_w, pend_u = [], []
                out.append(inst)
            assert not pend_w and not pend_u
            bb.instructions[:] = out
    return removed


TRACE = os.environ.get("KERNEL_TRACE", "0") == "1"
LAST_RESULTS = None


def _register_ntff_hook():
    """The image's antenv lacks axon_hooks; inject it and register the ctypes
    NTFF profile hook so run_bass_kernel_spmd(trace=True) works under axon."""
    import types

    if "antenv.axon_hooks" not in sys.modules:
        mod = types.ModuleType("antenv.axon_hooks")
        mod._hook = None

        def set_axon_ntff_profile_hook(h, _mod=mod):
            _mod._hook = h

        def get_axon_ntff_profile_hook(_mod=mod):
            return _mod._hook

        mod.set_axon_ntff_profile_hook = set_axon_ntff_profile_hook
        mod.get_axon_ntff_profile_hook = get_axon_ntff_profile_hook
        sys.modules["antenv.axon_hooks"] = mod
        import antenv

        antenv.axon_hooks = mod
    mod = sys.modules["antenv.axon_hooks"]
    if mod.get_axon_ntff_profile_hook() is None:
        try:
            from trn_agent_boot.trn_boot import _ntff_profile_via_ctypes

            mod.set_axon_ntff_profile_hook(
                _ntff_profile_via_ctypes("/opt/axon/libaxon_pjrt.so")
            )
        except Exception:
            pass


# ---------------------------------------------------------------- program
def build_program(n_st=N_ST, split_waits=True, warmup=20):
    """One SPMD NeuronCore program; all 8 cores run it on their own shard."""
    nc = bass.Bass("TRN2", target_bir_lowering=False, debug=False,
                   num_devices=N_CORES)

    xb = nc.dram_tensor("xb", [n_st, 96, ST_COLS], BF16,
                        kind="ExternalInput").ap()
    w1 = nc.dram_tensor("w1blk", [97, 128], BF16, kind="ExternalInput").ap()
    w2 = nc.dram_tensor("w2blk", [128, 64], BF16, kind="ExternalInput").ap()
    w3 = nc.dram_tensor("w3blk", [128, 64], BF16, kind="ExternalInput").ap()
    w4 = nc.dram_tensor("w4blk", [128, 64], BF16, kind="ExternalInput").ap()
    bv = nc.dram_tensor("bvecs", [128, 2], F32, kind="ExternalInput").ap()
    out = nc.dram_tensor("out", [n_st // 2, 128, FD], BF16,
                         kind="ExternalOutput").ap()

    Relu = mybir.ActivationFunctionType.Relu
    ADD = mybir.AluOpType.add
    MAX = mybir.AluOpType.max

    with tile.TileContext(nc) as tc:
        with (
            tc.tile_pool(name="consts", bufs=1) as cpool,
            tc.tile_pool(name="xin", bufs=1) as xpool,
            tc.tile_pool(name="zr", bufs=1) as zrpool,
            tc.tile_pool(name="ps", bufs=1, space="PSUM") as pspool,
        ):
            # fixed SBUF buffers
            xts = [xpool.tile([128, ST_COLS], BF16, tag=f"x{k}", name=f"x{k}")
                   for k in range(4)]
            z1rA = [zrpool.tile([128, 1024], BF16, tag=f"z1rA{k}",
                                name=f"z1rA{k}") for k in range(2)]
            z1rB = [zrpool.tile([128, 1024], BF16, tag=f"z1rB{k}",
                                name=f"z1rB{k}") for k in range(2)]
            z2r = [zrpool.tile([128, 1024], BF16, tag=f"z2r{k}",
                               name=f"z2r{k}") for k in range(2)]
            z3r = [zrpool.tile([128, 512], BF16, tag=f"z3r{k}",
                               name=f"z3r{k}") for k in range(2)]
            z4r = [zrpool.tile([128, 512], BF16, tag=f"z4r{k}",
                               name=f"z4r{k}") for k in range(2)]

            # fixed PSUM tiles (8 banks exactly)
            z1a = pspool.tile([128, 1024], F32, tag="z1a", name="z1a")
            z1b = pspool.tile([128, 1024], F32, tag="z1b", name="z1b")
            z2p = pspool.tile([128, 1024], F32, tag="z2p", name="z2p")
            z3p = pspool.tile([128, 512], F32, tag="z3p", name="z3p")
            z4p = pspool.tile([128, 512], F32, tag="z4p", name="z4p")

            # ones partitions (L1 bias row): just partition 96 of each
            # x buffer (16-aligned, so gpsimd can address it)
            for xt in xts:
                nc.gpsimd.memset(xt[96:97, :], 1.0)

            def load_x(s):
                xt = xts[s % 4]
                nc.sync.dma_start(xt[0:96, :], xb[s])
                return xt

            for s in range(min(3, n_st)):
                load_x(s)
            w1t = cpool.tile([97, 128], BF16, tag="w1")
            nc.sync.dma_start(w1t[:], w1[:])
            w2t = cpool.tile([128, 64], BF16, tag="w2")
            nc.sync.dma_start(w2t[:], w2[:])
            w3t = cpool.tile([128, 64], BF16, tag="w3")
            nc.sync.dma_start(w3t[:], w3[:])
            w4t = cpool.tile([128, 64], BF16, tag="w4")
            nc.sync.dma_start(w4t[:], w4[:])
            bvt = cpool.tile([128, 2], F32, tag="bv")
            nc.sync.dma_start(bvt[:], bv[:])
            b7v, b8v = bvt[:, 0:1], bvt[:, 1:2]

            # PE warmup against the HAM clock gate, into z1a's first bank
            if warmup:
                wscr = cpool.tile([32, 640], BF16, tag="wscr")
                nc.gpsimd.memset(wscr[:], 0.0)
                for _ in range(warmup):
                    nc.tensor.matmul(z1a[0:128, 0:512], wscr[0:32, 0:128],
                                     wscr[0:32, 128:640], start=True,
                                     stop=True)

            for it in range(n_st + 3):
                # L1 for supertile s1: 4 matmuls, one shared stationary
                if it < n_st:
                    s1 = it
                    if s1 + 3 < n_st:
                        load_x(s1 + 3)
                    xt = xts[s1 % 4]
                    for c in range(4):
                        zt, c0 = (z1a, c * 512) if c < 2 else (z1b,
                                                               (c - 2) * 512)
                        nc.tensor.matmul(
                            zt[:, c0 : c0 + 512],
                            w1t[:],
                            xt[0:97, 512 * c : 512 * c + 512],
                            start=True, stop=True,
                        )
                    if s1 == n_st - 1:
                        # fine-grained for the drain tail: L2 chunks start
                        # as soon as their 512-col slice is evacuated
                        for h in range(2):
                            nc.vector.tensor_scalar_max(
                                z1rA[s1 % 2][:, h * 512 : h * 512 + 512],
                                z1a[:, h * 512 : h * 512 + 512], 0.0)
                            nc.scalar.activation(
                                z1rB[s1 % 2][:, h * 512 : h * 512 + 512],
                                z1b[:, h * 512 : h * 512 + 512], Relu)
                    else:
                        nc.vector.tensor_scalar_max(z1rA[s1 % 2][:], z1a[:],
                                                    0.0)
                        nc.scalar.activation(z1rB[s1 % 2][:], z1b[:], Relu)

                # L2 for supertile s2 (j outer so identical LDWs are
                # consecutive and dedupe)
                if 0 <= it - 1 < n_st:
                    s2 = it - 1
                    vA, vB = z1rA[s2 % 2], z1rB[s2 % 2]
                    for j in range(2):
                        for half in range(2):
                            src = vA if half == 0 else vB
                            nc.tensor.matmul(
                                z2p[j * 64 : j * 64 + 64,
                                    half * 512 : half * 512 + 512],
                                w2t[:],
                                src[:, j * 512 : j * 512 + 512],
                                start=True, stop=True,
                            )
                    if s2 == n_st - 1:
                        for h in range(2):
                            nc.scalar.activation(
                                z2r[s2 % 2][:, h * 512 : h * 512 + 512],
                                z2p[:, h * 512 : h * 512 + 512], Relu,
                                bias=b7v, scale=1.0)
                    else:
                        nc.scalar.activation(z2r[s2 % 2][:], z2p[:], Relu,
                                             bias=b7v, scale=1.0)

                # L3 for supertile s3
                if 0 <= it - 2 < n_st:
                    s3 = it - 2
                    v2 = z2r[s3 % 2]
                    for u in range(2):
                        nc.tensor.matmul(
                            z3p[u * 64 : u * 64 + 64, :],
                            w3t[:],
                            v2[:, u * 512 : u * 512 + 512],
                            start=True, stop=True,
                        )
                    nc.vector.tensor_scalar(z3r[s3 % 2][:], z3p[:], b8v, 0.0,
                                            ADD, MAX)

                # L4 for supertile s4; supertile pairs share one PSUM bank
                if 0 <= it - 3 < n_st:
                    s4 = it - 3
                    half4 = (s4 % 2) * 64
                    nc.tensor.matmul(z4p[half4 : half4 + 64, :], w4t[:],
                                     z3r[s4 % 2][:], start=True, stop=True)
                    if s4 % 2 == 1:
                        zo = z4r[(s4 // 2) % 2]
                        nc.vector.tensor_copy(zo[:], z4p[:])
                        nc.sync.dma_start(out[s4 // 2], zo[:])

    _dedupe_ldweights(nc)
    if split_waits:
        _split_multi_waits(nc)
    return nc


# ---------------------------------------------------------------- host side
def _block_weights(W1, b1, W7, W8, W9):
    w1blk = np.zeros((97, 128), np.float32)
    for r in range(16):
        w1blk[r * 6 : r * 6 + 6, r * 8 : r * 8 + 8] = W1.T
    w1blk[96, :] = np.tile(b1, 16)
    w2blk = np.zeros((128, 64), np.float32)
    for r in range(16):
        w2blk[r * 8 : r * 8 + 8, r * 4 : r * 4 + 4] = W7.T
    w3blk = np.zeros((128, 64), np.float32)
    for h in range(2):
        for r in range(16):
            w3blk[h * 64 + r * 4 : h * 64 + r * 4 + 4,
                  h * 32 + r * 2 : h * 32 + r * 2 + 2] = W8.T
    w4blk = np.zeros((128, 64), np.float32)
    for q in range(2):
        for h in range(2):
            for r in range(16):
                w4blk[q * 64 + h * 32 + r * 2 : q * 64 + h * 32 + r * 2 + 2,
                      q * 32 + h * 16 + r] = W9.T[:, 0]
    return w1blk, w2blk, w3blk, w4blk


def kernel(x, W1, b1, W7, b7, W8, b8, W9, b9):
    import ml_dtypes

    x = np.ascontiguousarray(np.asarray(x, dtype=np.float32))
    W1, b1 = np.asarray(W1, np.float32), np.asarray(b1, np.float32)
    W7, b7 = np.asarray(W7, np.float32), np.asarray(b7, np.float32)
    W8, b8 = np.asarray(W8, np.float32), np.asarray(b8, np.float32)
    W9, b9 = np.asarray(W9, np.float32), np.asarray(b9, np.float32)

    bf = ml_dtypes.bfloat16
    w1blk, w2blk, w3blk, w4blk = _block_weights(W1, b1, W7, W8, W9)
    p = np.arange(128)
    bvecs = np.stack([b7[p % 4], b8[p % 2]], axis=1).astype(np.float32)

    # [N,6] -> per core [n_st, 96, st_cols]: block layout, partition = r*6+k
    xbh = (
        x.reshape(N_CORES, N_ST, ST_COLS, G, 6)
        .transpose(0, 1, 3, 4, 2)
        .reshape(N_CORES, N_ST, 96, ST_COLS)
        .astype(bf)
    )
    xbh = np.ascontiguousarray(xbh)

    nc = build_program()
    in_maps = [
        {
            "xb": xbh[c],
            "w1blk": w1blk.astype(bf),
            "w2blk": w2blk.astype(bf),
            "w3blk": w3blk.astype(bf),
            "w4blk": w4blk.astype(bf),
            "bvecs": bvecs,
        }
        for c in range(N_CORES)
    ]
    kwargs = {}
    if TRACE:
        _register_ntff_hook()
        kwargs["trace"] = True
    res = run_bass_kernel_spmd(nc, in_maps, list(range(N_CORES)), **kwargs)
    global LAST_RESULTS
    LAST_RESULTS = res

    # out[c] is [8, 128, 512] raw z4 (bf16);
    # row = ((((P*2+so)*2+q)*2+h)*512+c)*16+r. Host: +b9, softplus, fp32.
    outs = []
    for c in range(N_CORES):
        arr = np.asarray(res.results[c]["out"]).astype(np.float32)
        arr = (
            arr.reshape(N_ST // 2, 2, 2, 2, G, FD)
            .transpose(0, 1, 2, 3, 5, 4)
            .reshape(ROWS_PER_CORE)
        )
        outs.append(arr)
    z4 = np.concatenate(outs).astype(np.float64) + float(b9[0])
    y = np.logaddexp(0.0, z4).astype(np.float32)
    return np.ascontiguousarray(y.reshape(N_ROWS, 1))
